# revision 2
# baseline (speedup 1.0000x reference)
"""Trainium2 Bass kernel for nn_Conv2d_20590073217670.

Conv2d: input [32,64,64,64] (NCHW), weight [576,128] (unfold layout:
row = ci*9 + a*3 + b for tap (a,b)), bias [1,128,1,1], stride 1, pad 1.
Output [32,128,64,64].

Strategy: data-parallel over batch — 4 images per NeuronCore, 8 cores.
Per image, implicit GEMM: out[co, y, x] = sum_{a,b,ci} W[ci,a,b,co] *
img[ci, y+a-1, x+b-1].  SBUF holds the image on partitions 0:64 and a
one-row-down shifted copy on partitions 64:128, so a single K=128
matmul accumulates two vertical taps (a, a+1) at once.  The rounded
fp32r image is stored column-padded ([128, 64, 66], zero borders), so
every matmul is a full 64-wide slide satisfying the fp32r ISA
restrictions (even innermost count, 8B-aligned full-bank PSUM output).
Row borders are handled by restricting output rows (PSUM has_written
zero-fill keeps partial accumulation exact).  DVE produces all matmul
inputs (fp32->fp32r rounding) and evicts PSUM with a fused bias add.
"""
import sys

for _p in ("/opt/trn_rl_repo", "/root/.axon_site/_ro/trn_rl_repo"):
    if _p not in sys.path:
        sys.path.append(_p)

import numpy as np
from contextlib import ExitStack

import concourse.bacc as bacc
import concourse.tile as tile
from concourse import mybir
from concourse.bass_utils import run_bass_kernel_spmd

f32 = mybir.dt.float32
f32r = mybir.dt.bfloat16

N_CORES = 8
NB = 4  # images per core


def build_nc():
    nc = bacc.Bacc()
    x = nc.declare_dram_parameter("x", [NB, 64, 64, 64], f32, isOutput=False)
    w = nc.declare_dram_parameter("w", [576, 128], f32, isOutput=False)
    bias = nc.declare_dram_parameter("b", [128, 1], f32, isOutput=False)
    out = nc.declare_dram_parameter("out", [NB, 128, 64, 64], f32, isOutput=True)

    with tile.TileContext(nc) as tc, ExitStack() as ctx:
        const = ctx.enter_context(tc.tile_pool(name="const", bufs=1))
        xs_pool = ctx.enter_context(tc.tile_pool(name="xs", bufs=3))
        xr_pool = ctx.enter_context(tc.tile_pool(name="xr", bufs=3))
        ob_pool = ctx.enter_context(tc.tile_pool(name="ob", bufs=2))
        ps_pool = ctx.enter_context(tc.tile_pool(name="ps", bufs=8, space="PSUM"))

        # ---- weights: one [128, 9, 128] tile; partition p<64 holds channel
        # p's taps 0..8, partition 64+ci holds channel ci's taps 3..8 at
        # slots 0..5 (tap axis pre-shifted by -3).  Then the lhsT view
        # wr[:, t, :] pairs taps (t, t+3) across the partition halves:
        #   t in 0..2  -> taps (0,b) & (1,b)
        #   t in 3..5  -> taps (1,b) & (2,b)
        w3 = w[:].rearrange("(c t) m -> c t m", t=9)
        ws = const.tile([128, 9, 128], f32)
        wr = const.tile([128, 9, 128], f32r)
        bt = const.tile([128, 1], f32)
        zc = const.tile([128, 64, 1], f32)
        nc.sync.dma_start(out=ws[0:64, :, :], in_=w3)
        nc.sync.dma_start(out=ws[64:128, 0:6, :], in_=w3[:, 3:9, :])
        nc.sync.dma_start(out=bt[:], in_=bias[:])
        nc.vector.memset(zc[:], 0.0)
        nc.vector.tensor_copy(wr[0:64, :, :], ws[0:64, :, :])
        nc.vector.tensor_copy(wr[64:128, 0:6, :], ws[64:128, 0:6, :])

        for n in range(NB):
            xs = xs_pool.tile([128, 64, 64], f32)
            xr = xr_pool.tile([128, 64, 66], f32r)
            # image rows on partitions 0:64; one-row-down copy on 64:128
            nc.sync.dma_start(out=xs[0:64, :, :], in_=x[n])
            nc.sync.dma_start(out=xs[64:128, 0:63, :], in_=xs[0:64, 1:64, :])
            # fp32 -> fp32r rounding (DVE) into the column-padded layout;
            # upper-half row 63 is never read.  Zero border columns.
            nc.vector.tensor_copy(xr[:, 0:63, 1:65], xs[:, 0:63, :])
            nc.vector.tensor_copy(xr[0:64, 63, 1:65], xs[0:64, 63, :])
            nc.vector.tensor_copy(xr[:, :, 0:1], zc[:])
            nc.vector.tensor_copy(xr[:, :, 65:66], zc[:])

            osb = ob_pool.tile([128, 64, 64], f32)
            for blk in range(8):
                y0 = blk * 8
                P = ps_pool.tile([128, 8, 64], f32)
                if blk == 0:
                    pair_t, pr0 = 3, 0      # taps (1,2), rhs rows y0..y0+7
                else:
                    pair_t, pr0 = 0, y0 - 1  # taps (0,1), rhs rows y0-1..y0+6
                # b=1 first: full [8,64] coverage zero-fills the whole bank
                for k, b in enumerate((1, 0, 2)):
                    nc.tensor.matmul(
                        P[:, 0:8, :],
                        wr[:, pair_t + b, :],
                        xr[:, pr0:pr0 + 8, b:b + 64],
                        start=(k == 0), stop=False,
                    )
                # remaining vertical tap as K=64 single on partitions 0:64
                for k, b in enumerate((1, 0, 2)):
                    last = k == 2
                    if blk == 0:
                        # tap (0,b): out rows 1..7 read img rows 0..6
                        nc.tensor.matmul(
                            P[:, 1:8, :], wr[0:64, b, :],
                            xr[0:64, 0:7, b:b + 64],
                            start=False, stop=last,
                        )
                    elif blk == 7:
                        # tap (2,b): out rows 56..62 read img rows 57..63
                        nc.tensor.matmul(
                            P[:, 0:7, :], wr[0:64, 6 + b, :],
                            xr[0:64, 57:64, b:b + 64],
                            start=False, stop=last,
                        )
                    else:
                        nc.tensor.matmul(
                            P[:, 0:8, :], wr[0:64, 6 + b, :],
                            xr[0:64, y0 + 1:y0 + 9, b:b + 64],
                            start=False, stop=last,
                        )
                nc.vector.tensor_scalar_add(osb[:, y0:y0 + 8, :], P[:, :, :], bt[:])

            nc.sync.dma_start(out=out[n], in_=osb[:])

    nc.finalize()
    return nc


_NC = None


def _get_nc():
    global _NC
    if _NC is None:
        _NC = build_nc()
    return _NC


def kernel(**inputs) -> np.ndarray:
    x = np.ascontiguousarray(np.asarray(inputs["input"], dtype=np.float32))
    w = np.ascontiguousarray(np.asarray(inputs["weight"], dtype=np.float32))
    b = np.ascontiguousarray(
        np.asarray(inputs["bias"], dtype=np.float32).reshape(128, 1))
    nc = _get_nc()
    in_maps = [
        {"x": x[c * NB:(c + 1) * NB], "w": w, "b": b} for c in range(N_CORES)
    ]
    res = run_bass_kernel_spmd(nc, in_maps, list(range(N_CORES)))
    return np.concatenate([r["out"] for r in res.results], axis=0)



# revision 4
# speedup vs baseline: 1.6500x; 1.6500x over previous
"""Trainium2 Bass kernel for nn_Conv2d_20590073217670 (v3).

Conv2d: input [32,64,64,64] (NCHW), weight [576,128] (unfold layout:
row = ci*9 + a*3 + b for tap (a,b)), bias [1,128,1,1], stride 1, pad 1.
Output [32,128,64,64].  Data-parallel over batch: 4 images per core.

v3 strategy: process TWO images concurrently via PE row-group packing.
Image A's 64 channels sit on partitions 0:64, image B's on 64:128; the
bf16 weights are replicated on both halves.  Every conv tap (a,b) is a
K=64 matmul: the A-matmul (array rows 0:63) and B-matmul (rows 64:127)
run concurrently in the PE array and drain into different PSUM banks,
so each 512-column tap slot computes both images at once -- full
128-row array utilization with no K padding and no shifted image copy
(row taps are free-dim row offsets, column taps are free-dim column
offsets into a column-padded bf16 image).  bf16 operands keep the PE
HAM-warm at 2.4 GHz and enable fast weight loads; accumulation stays
fp32 in PSUM.  PSUM eviction (fused bias add) alternates between DVE
(image A) and ScalarE (image B) so neither engine is on the critical
path.
"""
import sys

for _p in ("/opt/trn_rl_repo", "/root/.axon_site/_ro/trn_rl_repo"):
    if _p not in sys.path:
        sys.path.append(_p)

import numpy as np
from contextlib import ExitStack

import concourse.bacc as bacc
import concourse.tile as tile
from concourse import mybir
from concourse.bass_utils import run_bass_kernel_spmd

f32 = mybir.dt.float32
bf16 = mybir.dt.bfloat16

N_CORES = 8
NB = 4  # images per core (processed as 2 concurrent pairs)


def build_nc():
    nc = bacc.Bacc()
    x = nc.declare_dram_parameter("x", [NB, 64, 64, 64], f32, isOutput=False)
    w = nc.declare_dram_parameter("w", [576, 128], f32, isOutput=False)
    bias = nc.declare_dram_parameter("b", [128, 1], f32, isOutput=False)
    out = nc.declare_dram_parameter("out", [NB, 128, 64, 64], f32, isOutput=True)

    with tile.TileContext(nc) as tc, ExitStack() as ctx:
        const = ctx.enter_context(tc.tile_pool(name="const", bufs=1))
        xs_pool = ctx.enter_context(tc.tile_pool(name="xs", bufs=2))
        xr_pool = ctx.enter_context(tc.tile_pool(name="xr", bufs=2))
        ob_pool = ctx.enter_context(tc.tile_pool(name="ob", bufs=4))
        ps_pool = ctx.enter_context(tc.tile_pool(name="ps", bufs=4, space="PSUM"))

        # ---- weights: [128, 9, 128] bf16, image-A copy on partitions 0:64
        # and an identical copy on 64:128 (feeds array rows 64:127 for the
        # concurrent image-B matmuls).
        w3 = w[:].rearrange("(c t) m -> c t m", t=9)
        ws = const.tile([128, 9, 128], f32)
        wr = const.tile([128, 9, 128], bf16)
        bt = const.tile([128, 1], f32)
        nc.sync.dma_start(out=ws[0:64, :, :], in_=w3)
        nc.sync.dma_start(out=ws[64:128, :, :], in_=w3)
        nc.vector.tensor_copy(wr[:, :, :], ws[:, :, :])
        nc.sync.dma_start(out=bt[:], in_=bias[:])

        for pair in range(NB // 2):
            nA, nB = 2 * pair, 2 * pair + 1
            # stage both images fp32: A -> partitions 0:64, B -> 64:128
            xs = xs_pool.tile([128, 64, 64], f32)
            nc.sync.dma_start(out=xs[0:64, :, :], in_=x[nA])
            nc.sync.dma_start(out=xs[64:128, :, :], in_=x[nB])
            # bf16 image, column-padded: img col c at xr col c+2; zero
            # borders at cols 1 and 66 (cols 0/67 are alignment pad, never
            # read).  Tap (a,b) reads xr cols b+1 .. b+64.
            xr = xr_pool.tile([128, 64, 68], bf16)
            nc.vector.memset(xr[:, :, 1:2], 0.0)
            nc.vector.memset(xr[:, :, 66:67], 0.0)
            nc.vector.tensor_copy(xr[:, :, 2:66], xs[:, :, :])

            osbA = ob_pool.tile([128, 64, 64], f32)
            osbB = ob_pool.tile([128, 64, 64], f32)
            for blk in range(8):
                y0 = blk * 8
                PA = ps_pool.tile([128, 8, 64], f32)
                PB = ps_pool.tile([128, 8, 64], f32)
                # tap order: a=1 first (full 8-row coverage zero-fills the
                # bank via start=True), then a=0 / a=2 with row limits at
                # the image borders.
                taps = [(1, 0), (1, 1), (1, 2), (0, 0), (0, 1), (0, 2),
                        (2, 0), (2, 1), (2, 2)]
                for k, (a, b) in enumerate(taps):
                    t = 3 * a + b
                    # out rows y0+r0 .. y0+r1, reading img rows y0+r+a-1
                    r0 = max(0, 1 - (y0 + a))          # a=0, blk=0 -> 1
                    r1 = min(8, 64 - (y0 + a - 1) )    # a=2, blk=7 -> 7
                    ir0 = y0 + r0 + a - 1
                    start, stop = k == 0, k == len(taps) - 1
                    nc.tensor.matmul(
                        PA[:, r0:r1, :], wr[0:64, t, :],
                        xr[0:64, ir0:ir0 + (r1 - r0), b + 1:b + 65],
                        start=start, stop=stop,
                    )
                    nc.tensor.matmul(
                        PB[:, r0:r1, :], wr[64:128, t, :],
                        xr[64:128, ir0:ir0 + (r1 - r0), b + 1:b + 65],
                        start=start, stop=stop,
                    )
                # fused bias add + PSUM->SBUF on two engines in parallel
                nc.vector.tensor_scalar_add(osbA[:, y0:y0 + 8, :], PA[:, :, :], bt[:])
                nc.scalar.activation(osbB[:, y0:y0 + 8, :], PB[:, :, :],
                                     mybir.ActivationFunctionType.Identity,
                                     bias=bt[:], scale=1.0)

            nc.sync.dma_start(out=out[nA], in_=osbA[:])
            nc.sync.dma_start(out=out[nB], in_=osbB[:])

    nc.finalize()
    return nc


_NC = None


def _get_nc():
    global _NC
    if _NC is None:
        _NC = build_nc()
    return _NC


def kernel(**inputs) -> np.ndarray:
    x = np.ascontiguousarray(np.asarray(inputs["input"], dtype=np.float32))
    w = np.ascontiguousarray(np.asarray(inputs["weight"], dtype=np.float32))
    b = np.ascontiguousarray(
        np.asarray(inputs["bias"], dtype=np.float32).reshape(128, 1))
    nc = _get_nc()
    in_maps = [
        {"x": x[c * NB:(c + 1) * NB], "w": w, "b": b} for c in range(N_CORES)
    ]
    res = run_bass_kernel_spmd(nc, in_maps, list(range(N_CORES)))
    return np.concatenate([r["out"] for r in res.results], axis=0)


# revision 5
# speedup vs baseline: 1.7481x; 1.0594x over previous
"""Trainium2 Bass kernel for nn_Conv2d_20590073217670 (v4).

Conv2d: input [32,64,64,64] (NCHW), weight [576,128] (unfold layout:
row = ci*9 + a*3 + b for tap (a,b)), bias [1,128,1,1], stride 1, pad 1.
Output [32,128,64,64].  Data-parallel over batch: 4 images per core.

Strategy: process TWO images concurrently via PE row-group packing.
Image A's 64 channels sit on partitions 0:64, image B's on 64:128; the
bf16 weights are replicated on both halves.  Every conv tap (a,b) is a
K=64 matmul: the A-matmul (array rows 0:63) and B-matmul (rows 64:127)
run concurrently in the PE array and drain into different PSUM banks,
so each 512-column tap slot computes both images at once -- full
128-row array utilization with no K padding and no shifted image copy
(row/column taps are free-dim AP offsets into a column-padded bf16
image).  bf16 operands keep the PE HAM-warm at 2.4 GHz and enable fast
weight loads; accumulation stays fp32 in PSUM.

Pipeline: inputs stream in 16-row chunks (all chunk DMAs issued
upfront for queue depth), GpSimd casts each chunk fp32->bf16 so the
first matmuls start as soon as chunk 0 lands; PSUM eviction (fused
bias add) alternates DVE (image A) / ScalarE (image B); outputs drain
per 16-row group so the final DMA tail is one small transfer.
"""
import sys

for _p in ("/opt/trn_rl_repo", "/root/.axon_site/_ro/trn_rl_repo"):
    if _p not in sys.path:
        sys.path.append(_p)

import numpy as np
from contextlib import ExitStack

import concourse.bacc as bacc
import concourse.tile as tile
from concourse import mybir
from concourse.bass_utils import run_bass_kernel_spmd

f32 = mybir.dt.float32
bf16 = mybir.dt.bfloat16

N_CORES = 8
NB = 4  # images per core (processed as 2 concurrent pairs)


def build_nc():
    nc = bacc.Bacc()
    x = nc.declare_dram_parameter("x", [NB, 64, 64, 64], f32, isOutput=False)
    w = nc.declare_dram_parameter("w", [576, 128], f32, isOutput=False)
    bias = nc.declare_dram_parameter("b", [128, 1], f32, isOutput=False)
    out = nc.declare_dram_parameter("out", [NB, 128, 64, 64], f32, isOutput=True)

    with tile.TileContext(nc) as tc, ExitStack() as ctx:
        const = ctx.enter_context(tc.tile_pool(name="const", bufs=1))
        xs_pool = ctx.enter_context(tc.tile_pool(name="xs", bufs=2))
        xr_pool = ctx.enter_context(tc.tile_pool(name="xr", bufs=2))
        ob_pool = ctx.enter_context(tc.tile_pool(name="ob", bufs=6))
        ps_pool = ctx.enter_context(tc.tile_pool(name="ps", bufs=4, space="PSUM"))

        # ---- weights: [128, 9, 128] bf16, image-A copy on partitions 0:64
        # and an identical copy on 64:128 (feeds array rows 64:127 for the
        # concurrent image-B matmuls).
        w3 = w[:].rearrange("(c t) m -> c t m", t=9)
        ws = const.tile([128, 9, 128], f32)
        wr = const.tile([128, 9, 128], bf16)
        bt = const.tile([128, 1], f32)
        nc.sync.dma_start(out=ws[0:64, :, :], in_=w3)
        nc.sync.dma_start(out=ws[64:128, :, :], in_=w3)
        nc.sync.dma_start(out=bt[:], in_=bias[:])
        nc.vector.tensor_copy(wr[:, :, :], ws[:, :, :])

        # ---- stage both pairs' inputs upfront in 16-row chunks: one
        # 128-partition DMA per chunk (imgA channels -> partitions 0:64,
        # imgB -> 64:128), queued early for DMA-engine depth.
        xs_t, xr_t = [], []
        for pair in range(NB // 2):
            nA = 2 * pair
            xp = x[nA:nA + 2].rearrange("n c h w -> (n c) h w")
            xs = xs_pool.tile([128, 64, 64], f32)
            for ch in range(4):
                r = slice(16 * ch, 16 * ch + 16)
                nc.sync.dma_start(out=xs[:, r, :], in_=xp[:, r, :])
            xs_t.append(xs)

        for pair in range(NB // 2):
            # bf16 image, column-padded: img col c at xr col c+2; zero
            # borders at cols 1 and 66 (cols 0/67 are alignment pad,
            # never read).  Tap (a,b) reads xr cols b+1 .. b+64.
            xr = xr_pool.tile([128, 64, 68], bf16)
            nc.gpsimd.memset(xr[:, :, 1:2], 0.0)
            nc.gpsimd.memset(xr[:, :, 66:67], 0.0)
            for ch in range(4):
                r = slice(16 * ch, 16 * ch + 16)
                nc.gpsimd.tensor_copy(xr[:, r, 2:66], xs_t[pair][:, r, :])
            xr_t.append(xr)

        taps = [(1, 0), (1, 1), (1, 2), (0, 0), (0, 1), (0, 2),
                (2, 0), (2, 1), (2, 2)]
        for pair in range(NB // 2):
            nA, nB = 2 * pair, 2 * pair + 1
            xr = xr_t[pair]
            for grp in range(4):  # 16-row output groups
                osbA = ob_pool.tile([128, 16, 64], f32)
                osbB = ob_pool.tile([128, 16, 64], f32)
                for half in range(2):
                    blk = 2 * grp + half
                    y0 = blk * 8
                    g0 = half * 8
                    PA = ps_pool.tile([128, 8, 64], f32)
                    PB = ps_pool.tile([128, 8, 64], f32)
                    # tap order: a=1 first (full 8-row coverage zero-fills
                    # the bank via start=True), then a=0 / a=2 with row
                    # limits at the image borders.
                    for k, (a, b) in enumerate(taps):
                        t = 3 * a + b
                        # out rows y0+r0 .. y0+r1, reading img row y0+r+a-1
                        r0 = max(0, 1 - (y0 + a))
                        r1 = min(8, 64 - (y0 + a - 1))
                        ir0 = y0 + r0 + a - 1
                        start, stop = k == 0, k == len(taps) - 1
                        nc.tensor.matmul(
                            PA[:, r0:r1, :], wr[0:64, t, :],
                            xr[0:64, ir0:ir0 + (r1 - r0), b + 1:b + 65],
                            start=start, stop=stop,
                        )
                        nc.tensor.matmul(
                            PB[:, r0:r1, :], wr[64:128, t, :],
                            xr[64:128, ir0:ir0 + (r1 - r0), b + 1:b + 65],
                            start=start, stop=stop,
                        )
                    # fused bias add + PSUM->SBUF on two engines in parallel
                    nc.vector.tensor_scalar_add(
                        osbA[:, g0:g0 + 8, :], PA[:, :, :], bt[:])
                    nc.scalar.activation(
                        osbB[:, g0:g0 + 8, :], PB[:, :, :],
                        mybir.ActivationFunctionType.Identity,
                        bias=bt[:], scale=1.0)
                yg = slice(16 * grp, 16 * grp + 16)
                nc.sync.dma_start(out=out[nA][:, yg, :], in_=osbA[:])
                nc.sync.dma_start(out=out[nB][:, yg, :], in_=osbB[:])

    nc.finalize()
    return nc


_NC = None


def _get_nc():
    global _NC
    if _NC is None:
        _NC = build_nc()
    return _NC


def kernel(**inputs) -> np.ndarray:
    x = np.ascontiguousarray(np.asarray(inputs["input"], dtype=np.float32))
    w = np.ascontiguousarray(np.asarray(inputs["weight"], dtype=np.float32))
    b = np.ascontiguousarray(
        np.asarray(inputs["bias"], dtype=np.float32).reshape(128, 1))
    nc = _get_nc()
    in_maps = [
        {"x": x[c * NB:(c + 1) * NB], "w": w, "b": b} for c in range(N_CORES)
    ]
    res = run_bass_kernel_spmd(nc, in_maps, list(range(N_CORES)))
    return np.concatenate([r["out"] for r in res.results], axis=0)


# revision 7
# speedup vs baseline: 2.0498x; 1.1726x over previous
"""Trainium2 Bass kernel for nn_Conv2d_20590073217670 (v5).

Conv2d: input [32,64,64,64] (NCHW), weight [576,128] (unfold layout:
row = ci*9 + a*3 + b for tap (a,b)), bias [1,128,1,1], stride 1, pad 1.
Output [32,128,64,64].  Data-parallel over batch: 4 images per core.

Strategy: process TWO images concurrently via PE row-group packing.
Image A's 64 channels sit on partitions 0:64, image B's on 64:128; the
bf16 weights are replicated on both halves.  Every conv tap (a,b) is a
K=64 matmul: the A-matmul (array rows 0:63) and B-matmul (rows 64:127)
run concurrently in the PE array and drain into different PSUM banks,
so each 512-column tap slot computes both images at once -- full
128-row array utilization with no K padding and no shifted image copy
(row/column taps are free-dim AP offsets into a column-padded bf16
image).  bf16 operands keep the PE HAM-warm at 2.4 GHz and enable fast
weight loads; accumulation stays fp32 in PSUM.

Pipeline: inputs stream in 16-row chunks alternating between the two
hardware DMA queues (Sync and Scalar) to double the in-flight HBM read
packets; DVE casts each chunk fp32->bf16 (pair-1 casts interleaved
between pair-0 PSUM evictions to keep the DVE FIFO from head-of-line
blocking); eviction (fused bias add) alternates DVE (image A) /
ScalarE (image B); outputs drain per 16-row group, also alternating
queues, so the final DMA tail is one small transfer.
"""
import sys

for _p in ("/opt/trn_rl_repo", "/root/.axon_site/_ro/trn_rl_repo"):
    if _p not in sys.path:
        sys.path.append(_p)

import numpy as np
from contextlib import ExitStack

import concourse.bacc as bacc
import concourse.tile as tile
from concourse import mybir
from concourse.bass_utils import run_bass_kernel_spmd

f32 = mybir.dt.float32
bf16 = mybir.dt.bfloat16

N_CORES = 8
NB = 4  # images per core (processed as 2 concurrent pairs)

TAPS = [(1, 0), (1, 1), (1, 2), (0, 0), (0, 1), (0, 2),
        (2, 0), (2, 1), (2, 2)]


def build_nc():
    nc = bacc.Bacc()
    x = nc.declare_dram_parameter("x", [NB, 64, 64, 64], f32, isOutput=False)
    w = nc.declare_dram_parameter("w", [576, 128], f32, isOutput=False)
    bias = nc.declare_dram_parameter("b", [128, 1], f32, isOutput=False)
    out = nc.declare_dram_parameter("out", [NB, 128, 64, 64], f32, isOutput=True)

    with tile.TileContext(nc) as tc, ExitStack() as ctx:
        const = ctx.enter_context(tc.tile_pool(name="const", bufs=1))
        xs_pool = ctx.enter_context(tc.tile_pool(name="xs", bufs=2))
        xr_pool = ctx.enter_context(tc.tile_pool(name="xr", bufs=2))
        ob_pool = ctx.enter_context(tc.tile_pool(name="ob", bufs=6))
        ps_pool = ctx.enter_context(tc.tile_pool(name="ps", bufs=4, space="PSUM"))

        # ---- weights: [128, 9, 128] bf16, image-A copy on partitions 0:64
        # and an identical copy on 64:128 (feeds array rows 64:127 for the
        # concurrent image-B matmuls).
        w3 = w[:].rearrange("(c t) m -> c t m", t=9)
        ws = const.tile([128, 9, 128], f32)
        wr = const.tile([128, 9, 128], bf16)
        bt = const.tile([128, 1], f32)
        nc.sync.dma_start(out=ws[0:64, :, :], in_=w3)
        nc.scalar.dma_start(out=ws[64:128, :, :], in_=w3)
        nc.scalar.dma_start(out=bt[:], in_=bias[:])

        # ---- stage both pairs' inputs upfront in 16-row chunks: one
        # 128-partition DMA per chunk (imgA channels -> partitions 0:64,
        # imgB -> 64:128), alternating the two hardware DMA queues for
        # read-packet depth.
        xs_t, xr_t = [], []
        for pair in range(NB // 2):
            nA = 2 * pair
            xp = x[nA:nA + 2].rearrange("n c h w -> (n c) h w")
            xs = xs_pool.tile([128, 64, 64], f32)
            for ch in range(4):
                r = slice(16 * ch, 16 * ch + 16)
                eng = nc.sync if (ch % 2 == 0) else nc.scalar
                eng.dma_start(out=xs[:, r, :], in_=xp[:, r, :])
            xs_t.append(xs)
            xr_t.append(xr_pool.tile([128, 64, 68], bf16, name=f"xr{pair}"))

        nc.vector.tensor_copy(wr[:, :, :], ws[:, :, :])

        # pair-0 image prep on DVE: bf16 cast into the column-padded
        # layout (img col c at xr col c+2; tap (a,b) reads cols b+1..b+64;
        # border cols 1 and 66 are zero, cols 0/67 alignment pad).
        def prep_chunk(pair, ch):
            r = slice(16 * ch, 16 * ch + 16)
            nc.vector.tensor_copy(xr_t[pair][:, r, 2:66], xs_t[pair][:, r, :])

        for pair in range(NB // 2):
            nc.vector.memset(xr_t[pair][:, :, 1:2], 0.0)
            nc.vector.memset(xr_t[pair][:, :, 66:67], 0.0)
        for ch in range(4):
            prep_chunk(0, ch)

        for pair in range(NB // 2):
            nA, nB = 2 * pair, 2 * pair + 1
            xr = xr_t[pair]
            for grp in range(4):  # 16-row output groups
                osbA = ob_pool.tile([128, 16, 64], f32)
                osbB = ob_pool.tile([128, 16, 64], f32)
                for half in range(2):
                    blk = 2 * grp + half
                    y0 = blk * 8
                    g0 = half * 8
                    PA = ps_pool.tile([128, 8, 64], f32)
                    PB = ps_pool.tile([128, 8, 64], f32)
                    # tap order: a=1 first (full 8-row coverage zero-fills
                    # the bank via start=True), then a=0 / a=2 with row
                    # limits at the image borders.
                    for k, (a, b) in enumerate(TAPS):
                        t = 3 * a + b
                        # out rows y0+r0 .. y0+r1, reading img row y0+r+a-1
                        r0 = max(0, 1 - (y0 + a))
                        r1 = min(8, 64 - (y0 + a - 1))
                        ir0 = y0 + r0 + a - 1
                        start, stop = k == 0, k == len(TAPS) - 1
                        nc.tensor.matmul(
                            PA[:, r0:r1, :], wr[0:64, t, :],
                            xr[0:64, ir0:ir0 + (r1 - r0), b + 1:b + 65],
                            start=start, stop=stop,
                        )
                        nc.tensor.matmul(
                            PB[:, r0:r1, :], wr[64:128, t, :],
                            xr[64:128, ir0:ir0 + (r1 - r0), b + 1:b + 65],
                            start=start, stop=stop,
                        )
                    # fused bias add + PSUM->SBUF on two engines in parallel
                    nc.vector.tensor_scalar_add(
                        osbA[:, g0:g0 + 8, :], PA[:, :, :], bt[:])
                    nc.scalar.activation(
                        osbB[:, g0:g0 + 8, :], PB[:, :, :],
                        mybir.ActivationFunctionType.Identity,
                        bias=bt[:], scale=1.0)
                    # interleave next pair's chunk casts into the DVE
                    # stream so they never block pair-0 evictions
                    if pair == 0 and blk in (2, 3, 4, 5):
                        prep_chunk(1, blk - 2)
                yg = slice(16 * grp, 16 * grp + 16)
                eng = nc.sync if grp % 2 == 0 else nc.scalar
                eng.dma_start(out=out[nA][:, yg, :], in_=osbA[:])
                eng2 = nc.scalar if grp % 2 == 0 else nc.sync
                eng2.dma_start(out=out[nB][:, yg, :], in_=osbB[:])

    nc.finalize()
    return nc


_NC = None


def _get_nc():
    global _NC
    if _NC is None:
        _NC = build_nc()
    return _NC


def kernel(**inputs) -> np.ndarray:
    x = np.ascontiguousarray(np.asarray(inputs["input"], dtype=np.float32))
    w = np.ascontiguousarray(np.asarray(inputs["weight"], dtype=np.float32))
    b = np.ascontiguousarray(
        np.asarray(inputs["bias"], dtype=np.float32).reshape(128, 1))
    nc = _get_nc()
    in_maps = [
        {"x": x[c * NB:(c + 1) * NB], "w": w, "b": b} for c in range(N_CORES)
    ]
    res = run_bass_kernel_spmd(nc, in_maps, list(range(N_CORES)))
    return np.concatenate([r["out"] for r in res.results], axis=0)


# revision 8
# speedup vs baseline: 2.1079x; 1.0283x over previous
"""Trainium2 Bass kernel for nn_Conv2d_20590073217670 (v6).

Conv2d: input [32,64,64,64] (NCHW), weight [576,128] (unfold layout:
row = ci*9 + a*3 + b for tap (a,b)), bias [1,128,1,1], stride 1, pad 1.
Output [32,128,64,64].  Data-parallel over batch: 4 images per core.

Strategy: process TWO images concurrently via PE row-group packing.
Image A's 64 channels sit on partitions 0:64, image B's on 64:128; the
bf16 weights are replicated on both halves.  Every conv tap (a,b) is a
K=64 matmul: the A-matmul (array rows 0:63) and B-matmul (rows 64:127)
run concurrently in the PE array and drain into different PSUM banks,
so each 512-column tap slot computes both images at once -- full
128-row array utilization with no K padding and no shifted image copy
(row/column taps are free-dim AP offsets into a column-padded bf16
image).  bf16 operands keep the PE HAM-warm at 2.4 GHz and enable fast
weight loads; accumulation stays fp32 in PSUM.

Pipeline: inputs stream in 8 row-chunks per pair (chunk k covers
exactly the rows block k needs, so matmuls start after one small
chunk), alternating between the two hardware DMA queues (Sync and
Scalar) for HBM read-packet depth; DVE casts each chunk fp32->bf16
(pair-1 casts interleaved between pair-0 PSUM evictions to keep the
DVE FIFO from head-of-line blocking); eviction (fused bias add)
alternates DVE (image A) / ScalarE (image B) and writes bf16 output
tiles -- the output is DMA'd to HBM as bf16 (half the write traffic)
and widened to fp32 on the host after the gather.
"""
import sys

for _p in ("/opt/trn_rl_repo", "/root/.axon_site/_ro/trn_rl_repo"):
    if _p not in sys.path:
        sys.path.append(_p)

import numpy as np
from contextlib import ExitStack

import concourse.bacc as bacc
import concourse.tile as tile
from concourse import mybir
from concourse.bass_utils import run_bass_kernel_spmd

f32 = mybir.dt.float32
bf16 = mybir.dt.bfloat16

N_CORES = 8
NB = 4  # images per core (processed as 2 concurrent pairs)

TAPS = [(1, 0), (1, 1), (1, 2), (0, 0), (0, 1), (0, 2),
        (2, 0), (2, 1), (2, 2)]
# input row-chunk bounds: chunk k ends at row 8k+9, so block k's taps
# (which read image rows up to 8k+8) wait only on chunks 0..k
CHUNKS = [0, 9, 17, 25, 33, 41, 49, 57, 64]


def build_nc():
    nc = bacc.Bacc()
    x = nc.declare_dram_parameter("x", [NB, 64, 64, 64], f32, isOutput=False)
    w = nc.declare_dram_parameter("w", [576, 128], f32, isOutput=False)
    bias = nc.declare_dram_parameter("b", [128, 1], f32, isOutput=False)
    out = nc.declare_dram_parameter("out", [NB, 128, 64, 64], bf16,
                                    isOutput=True)

    with tile.TileContext(nc) as tc, ExitStack() as ctx:
        const = ctx.enter_context(tc.tile_pool(name="const", bufs=1))
        xs_pool = ctx.enter_context(tc.tile_pool(name="xs", bufs=2))
        xr_pool = ctx.enter_context(tc.tile_pool(name="xr", bufs=2))
        ob_pool = ctx.enter_context(tc.tile_pool(name="ob", bufs=6))
        ps_pool = ctx.enter_context(tc.tile_pool(name="ps", bufs=4, space="PSUM"))

        # ---- weights: [128, 9, 128] bf16, image-A copy on partitions 0:64
        # and an identical copy on 64:128 (feeds array rows 64:127 for the
        # concurrent image-B matmuls).
        w3 = w[:].rearrange("(c t) m -> c t m", t=9)
        ws = const.tile([128, 9, 128], f32)
        wr = const.tile([128, 9, 128], bf16)
        bt = const.tile([128, 1], f32)
        nc.sync.dma_start(out=ws[0:64, :, :], in_=w3)
        nc.scalar.dma_start(out=ws[64:128, :, :], in_=w3)
        nc.scalar.dma_start(out=bt[:], in_=bias[:])

        # ---- stage both pairs' inputs upfront in row chunks: one
        # 128-partition DMA per chunk (imgA channels -> partitions 0:64,
        # imgB -> 64:128), alternating the two hardware DMA queues.
        xs_t, xr_t = [], []
        for pair in range(NB // 2):
            nA = 2 * pair
            xp = x[nA:nA + 2].rearrange("n c h w -> (n c) h w")
            xs = xs_pool.tile([128, 64, 64], f32)
            for ch in range(8):
                r = slice(CHUNKS[ch], CHUNKS[ch + 1])
                eng = nc.sync if (ch % 2 == 0) else nc.scalar
                eng.dma_start(out=xs[:, r, :], in_=xp[:, r, :])
            xs_t.append(xs)
            xr_t.append(xr_pool.tile([128, 64, 68], bf16, name=f"xr{pair}"))

        nc.vector.tensor_copy(wr[:, :, :], ws[:, :, :])

        # pair-0 image prep on DVE: bf16 cast into the column-padded
        # layout (img col c at xr col c+2; tap (a,b) reads cols b+1..b+64;
        # border cols 1 and 66 are zero, cols 0/67 alignment pad).
        def prep_chunk(pair, ch):
            r = slice(CHUNKS[ch], CHUNKS[ch + 1])
            nc.vector.tensor_copy(xr_t[pair][:, r, 2:66], xs_t[pair][:, r, :])

        for pair in range(NB // 2):
            nc.vector.memset(xr_t[pair][:, :, 1:2], 0.0)
            nc.vector.memset(xr_t[pair][:, :, 66:67], 0.0)
        for ch in range(8):
            prep_chunk(0, ch)

        for pair in range(NB // 2):
            nA, nB = 2 * pair, 2 * pair + 1
            xr = xr_t[pair]
            for grp in range(4):  # 16-row output groups
                osbA = ob_pool.tile([128, 16, 64], bf16)
                osbB = ob_pool.tile([128, 16, 64], bf16)
                for half in range(2):
                    blk = 2 * grp + half
                    y0 = blk * 8
                    g0 = half * 8
                    PA = ps_pool.tile([128, 8, 64], f32)
                    PB = ps_pool.tile([128, 8, 64], f32)
                    # tap order: a=1 first (full 8-row coverage zero-fills
                    # the bank via start=True), then a=0 / a=2 with row
                    # limits at the image borders.
                    for k, (a, b) in enumerate(TAPS):
                        t = 3 * a + b
                        # out rows y0+r0 .. y0+r1, reading img row y0+r+a-1
                        r0 = max(0, 1 - (y0 + a))
                        r1 = min(8, 64 - (y0 + a - 1))
                        ir0 = y0 + r0 + a - 1
                        start, stop = k == 0, k == len(TAPS) - 1
                        nc.tensor.matmul(
                            PA[:, r0:r1, :], wr[0:64, t, :],
                            xr[0:64, ir0:ir0 + (r1 - r0), b + 1:b + 65],
                            start=start, stop=stop,
                        )
                        nc.tensor.matmul(
                            PB[:, r0:r1, :], wr[64:128, t, :],
                            xr[64:128, ir0:ir0 + (r1 - r0), b + 1:b + 65],
                            start=start, stop=stop,
                        )
                    # fused bias add + PSUM->SBUF(bf16) on two engines
                    nc.vector.tensor_scalar_add(
                        osbA[:, g0:g0 + 8, :], PA[:, :, :], bt[:])
                    nc.scalar.activation(
                        osbB[:, g0:g0 + 8, :], PB[:, :, :],
                        mybir.ActivationFunctionType.Identity,
                        bias=bt[:], scale=1.0)
                    # interleave next pair's chunk casts into the DVE
                    # stream so they never block pair-0 evictions
                    if pair == 0:
                        prep_chunk(1, blk)
                yg = slice(16 * grp, 16 * grp + 16)
                eng = nc.sync if grp % 2 == 0 else nc.scalar
                eng.dma_start(out=out[nA][:, yg, :], in_=osbA[:])
                eng2 = nc.scalar if grp % 2 == 0 else nc.sync
                eng2.dma_start(out=out[nB][:, yg, :], in_=osbB[:])

    nc.finalize()
    return nc


_NC = None


def _get_nc():
    global _NC
    if _NC is None:
        _NC = build_nc()
    return _NC


def kernel(**inputs) -> np.ndarray:
    x = np.ascontiguousarray(np.asarray(inputs["input"], dtype=np.float32))
    w = np.ascontiguousarray(np.asarray(inputs["weight"], dtype=np.float32))
    b = np.ascontiguousarray(
        np.asarray(inputs["bias"], dtype=np.float32).reshape(128, 1))
    nc = _get_nc()
    in_maps = [
        {"x": x[c * NB:(c + 1) * NB], "w": w, "b": b} for c in range(N_CORES)
    ]
    res = run_bass_kernel_spmd(nc, in_maps, list(range(N_CORES)))
    return np.concatenate(
        [np.asarray(r["out"]).astype(np.float32) for r in res.results], axis=0)
